# revision 23
# baseline (speedup 1.0000x reference)
"""LoRADense (per-token adapter routing) Bass kernel for 8 Trainium2 NeuronCores.

Math (reference):
    base  = x @ kernel + bias                      # (N, F)
    a     = lora_a[adapter_ids]                    # (N, D, R) gather
    b     = lora_b[adapter_ids]                    # (N, R, F) gather
    lr    = einsum('nd,ndr->nr', x, a)             # (N, R)
    delta = einsum('nr,nrf->nf', lr, b)            # (N, F)
    out   = base + delta

Strategy (v5):
  - GLOBAL sort of all 8192 tokens by adapter id on the host; core c gets the
    contiguous sorted run [1024c, 1024(c+1)).  Within a core, each 512-token
    chunk sees only ~5 consecutive adapter ids, so the host gathers, per
    (core, chunk), one 128-row band (8 adapters; spc slabs in general) of the
    concatenated LoRA factors, re-based so the device program is identical on
    every core (SPMD-safe).
  - Everything runs in bf16 (f32 PSUM accumulation), output stored bf16.
  - Transposed compute: out^T[f, tok] so the moving operand is always the
    token axis (512-wide chunks) and every stationary 128x128 block streams
    512 tokens:
      stage A: lr[sr_band, tok] = A_band^T @ x  (accumulate over 8 D-slabs),
               masked per (sr row, token) on DVE -> bf16 lrm in SBUF.
      stage B: po[f_blk, tok]   = sum_k Wk^T @ x  +  B_band^T @ lrm
               (one PSUM group of 8+spc matmuls), then +bias (per-partition
               scalar) fused with the f32->bf16 convert, DMA to DRAM.
  - k-major schedule in f-block passes sized to the 8 PSUM banks; pass 0
    carries stage A.  The per-k data (A band | x slab | first W f-blocks) is
    packed into ONE DMA per k so the stream feeds pass 0 just-in-time; the
    remaining W f-blocks stream during pass 1.
  - Host un-permutes rows and upcasts to f32.
"""

import numpy as np
import ml_dtypes

import concourse.bacc as bacc
import concourse.bass as bass
import concourse.mybir as mybir
import concourse.tile as tile
from concourse.bass_utils import run_bass_kernel_spmd

# Problem constants (hardcoded per harness contract).
N = 8192          # tokens
D = 1024          # input dim
F = 1024          # output features
R = 16            # lora rank
S = 64            # adapter slots
SR = S * R        # 1024
NCORES = 8
NTOK = N // NCORES            # 1024 tokens per core
P = 128                       # partitions
KD = D // P                   # 8 contraction slabs over D
TCH = 512                     # moving-operand token chunk
NCH = NTOK // TCH             # 2 chunks per core

BF16 = ml_dtypes.bfloat16

# Toggles (test.py pokes these).
TRACE = False
LAST_RESULTS = None
LAST_IN_MAPS = None
LAST_NC = None
LAST_NS = None

JUNK = 7
_NC_CACHE = {}


def _passes(spc):
    """f-block passes + whether stage A rides in pass 0, given PSUM budget 8."""
    n_lr = NCH * spc
    if n_lr <= 8 - NCH:  # room for at least one f-block next to the lr banks
        g0 = (8 - n_lr) // NCH
        jgs = [tuple(range(g0))]
        a_in_pass0 = True
    else:
        jgs = []
        a_in_pass0 = False
        g0 = 0
    j = g0
    while j < KD:
        g = min(8 // NCH, KD - 1 - j) if j < KD - 1 else 1
        g = max(1, min(g, KD - j - 1 if KD - j > 1 else 1))
        jgs.append(tuple(range(j, j + g)))
        j += g
    return jgs, a_in_pass0


def _build_nc(spc):
    """Build the single-core Bass program (same program runs on all 8 cores).

    spc = LoRA slabs (128-row bands) per 512-token chunk; normally 1.
    """
    f32 = mybir.dt.float32
    bf16 = mybir.dt.bfloat16
    nsl = NCH * spc                 # total gathered slabs per core
    jgs, a_in_p0 = _passes(spc)
    nja = len(jgs[0]) if a_in_p0 else 0   # f-blocks packed with the k-stream
    ACW = nsl * P                   # A-band columns in the pack
    XO = ACW                        # x offset in the pack
    WO = ACW + NTOK                 # W offset in the pack
    PKW = WO + nja * P              # pack width (bf16 elements)
    NJB = KD - nja                  # f-blocks in the second W stream

    nc = bacc.Bacc("TRN2", target_bir_lowering=False, debug=False)

    # DRAM I/O. Layouts are pre-packed on the host so every DMA is a plain
    # contiguous [partition, free...] copy.
    pk = nc.dram_tensor("pk", [P, KD, PKW], bf16, kind="ExternalInput")
    wkb = nc.dram_tensor("wkb", [P, KD, NJB * P], bf16, kind="ExternalInput")
    bs = nc.dram_tensor("bs", [P, nsl, F], bf16, kind="ExternalInput")
    msk = nc.dram_tensor("msk", [P, spc, NTOK], bf16, kind="ExternalInput")
    bia = nc.dram_tensor("bia", [P, KD], f32, kind="ExternalInput")
    out_s = nc.dram_tensor("out_s", [KD, P, NTOK], bf16, kind="ExternalOutput")

    with tile.TileContext(nc) as tc:
        with (
            tc.tile_pool(name="const", bufs=1) as cpool,
            tc.tile_pool(name="work", bufs=4) as wpool,
            tc.tile_pool(name="accp", bufs=8, space="PSUM") as accp,
        ):
            # Just-in-time DMA stream: one pack per D-slab k feeds pass 0.
            pk_sb = cpool.tile([P, KD, PKW], bf16)
            nc.sync.dma_start(pk_sb[:, 0, :XO + TCH], pk[:, 0, :XO + TCH])
            nc.sync.dma_start(pk_sb[:, 0, XO + TCH:], pk[:, 0, XO + TCH:])
            for k in range(1, KD):
                nc.sync.dma_start(pk_sb[:, k], pk[:, k])
            msk_sb = cpool.tile([P, spc, NTOK], bf16)
            nc.sync.dma_start(msk_sb[:], msk[:])
            bia_sb = cpool.tile([P, KD], f32)
            nc.sync.dma_start(bia_sb[:], bia[:])
            bs_sb = cpool.tile([P, nsl, F], bf16)
            nc.sync.dma_start(bs_sb[:], bs[:])
            wkb_sb = cpool.tile([P, KD, NJB * P], bf16)
            for k in range(KD):
                nc.sync.dma_start(wkb_sb[:, k], wkb[:, k])

            def wblk(k, j):
                if j < nja:
                    return pk_sb[:, k, WO + j * P:WO + (j + 1) * P]
                return wkb_sb[:, k, (j - nja) * P:(j - nja + 1) * P]

            # Masked low-rank activations, bf16: [sr_p, chunk-band, tok]
            lrm_sb = cpool.tile([P, spc, NTOK], bf16)

            # Warm-up: keep the PE busy (and the HAM clock-gate ramping)
            # while the first input packs are still in flight.  The junk
            # accumulator borrows one accp slot and is released before the
            # last pass-0 group needs its bank.
            junk_sb = cpool.tile([P, P], bf16)
            nc.vector.memset(junk_sb[:], 0.0)
            jp = accp.tile([P, TCH], mybir.dt.float32, tag="acc", name="jp")
            for w in range(JUNK):
                nc.tensor.matmul(
                    jp[:, :P], junk_sb[:], junk_sb[:],
                    start=True, stop=True,
                )

            def stage_a(t, o, k, ps):
                tok = slice(t * TCH, (t + 1) * TCH)
                nc.tensor.matmul(
                    ps[:],
                    pk_sb[:, k, (t * spc + o) * P:(t * spc + o + 1) * P],
                    pk_sb[:, k, XO + t * TCH:XO + (t + 1) * TCH],
                    start=(k == 0),
                    stop=(k == KD - 1),
                )
                if k == KD - 1:
                    # msk[p, o, tok] = (lid[tok] == (o*128+p)//16), host-built
                    nc.vector.tensor_tensor(
                        lrm_sb[:, o, tok],
                        ps[:],
                        msk_sb[:, o, tok],
                        mybir.AluOpType.mult,
                    )

            obs = {}

            def close_group(t, j, po):
                tok = slice(t * TCH, (t + 1) * TCH)
                for o in range(spc):
                    nc.tensor.matmul(
                        po[:],
                        bs_sb[:, t * spc + o, j * P:(j + 1) * P],
                        lrm_sb[:, o, tok],
                        start=False,
                        stop=(o == spc - 1),
                    )
                if j not in obs:
                    obs[j] = wpool.tile([P, NTOK], bf16, tag="ob",
                                        name=f"ob_{j}")
                nc.any.tensor_scalar_add(obs[j][:, tok], po[:],
                                         bia_sb[:, j:j + 1])
                if j == KD - 1:
                    # last f-block: per-chunk DMA so the first half overlaps
                    # the final chunk's close + convert
                    nc.sync.dma_start(out_s[j, :, tok], obs[j][:, tok])
                elif t == NCH - 1:
                    nc.sync.dma_start(out_s[j], obs[j][:])

            run_a = a_in_p0
            if not a_in_p0:
                # Fallback: sequential stage A before the f-block passes.
                for t in range(NCH):
                    for o in range(spc):
                        ps = accp.tile([P, TCH], mybir.dt.float32, tag="acc",
                                       name=f"lr_{t}_{o}")
                        for k in range(KD):
                            stage_a(t, o, k, ps)

            for gi, jg in enumerate(jgs):
                last = gi == len(jgs) - 1
                pos = {}
                lrs = {}
                for t in range(NCH):
                    for j in jg:
                        pos[(t, j)] = accp.tile(
                            [P, TCH], mybir.dt.float32, tag="acc",
                            name=f"po_{t}_{j}")
                    if gi == 0 and run_a:
                        for o in range(spc):
                            lrs[(t, o)] = accp.tile(
                                [P, TCH], mybir.dt.float32, tag="acc",
                                name=f"lr_{t}_{o}")
                if last:
                    # t-major: the first chunk's close/convert/DMA overlaps
                    # the second chunk's matmuls, shortening the tail.
                    for t in range(NCH):
                        for k in range(KD):
                            for j in jg:
                                nc.tensor.matmul(
                                    pos[(t, j)][:],
                                    wblk(k, j),
                                    pk_sb[:, k,
                                          XO + t * TCH:XO + (t + 1) * TCH],
                                    start=(k == 0),
                                    stop=False,
                                )
                        for j in jg:
                            close_group(t, j, pos[(t, j)])
                    continue
                for k in range(KD):
                    for t in range(NCH):
                        if gi == 0 and run_a:
                            for o in range(spc):
                                stage_a(t, o, k, lrs[(t, o)])
                        for j in jg:
                            nc.tensor.matmul(
                                pos[(t, j)][:],
                                wblk(k, j),
                                pk_sb[:, k, XO + t * TCH:XO + (t + 1) * TCH],
                                start=(k == 0),
                                stop=False,
                            )
                for t in range(NCH):
                    for j in jg:
                        close_group(t, j, pos[(t, j)])

    nc.compile()
    return nc


def _get_nc(spc):
    key = (spc, JUNK)
    if key not in _NC_CACHE:
        _NC_CACHE[key] = _build_nc(spc)
    return _NC_CACHE[key]


def kernel(x, adapter_ids, kernel, bias, lora_a, lora_b):
    global LAST_RESULTS, LAST_IN_MAPS, LAST_NC, LAST_NS
    x = np.ascontiguousarray(np.asarray(x, dtype=np.float32))
    adapter_ids = np.asarray(adapter_ids)
    kernel_w = np.asarray(kernel, dtype=np.float32)
    bias = np.asarray(bias, dtype=np.float32)
    lora_a = np.asarray(lora_a, dtype=np.float32)
    lora_b = np.asarray(lora_b, dtype=np.float32)
    ids = adapter_ids.astype(np.int64)

    # Global stable sort by adapter id; each core gets a contiguous run.
    perm = np.argsort(ids, kind="stable")
    ids_s = ids[perm]
    xs_all = x[perm]

    # Per-(core, chunk) adapter band [a0, a0 + 8*spc).
    spans = []
    for cc in range(NCORES * NCH):
        blk = ids_s[cc * TCH:(cc + 1) * TCH]
        spans.append(int(blk.max()) - int(blk.min()) + 1)
    spc = FORCE_SPC or max(1, int(np.ceil(max(spans) / 8)))
    a0s = []
    for cc in range(NCORES * NCH):
        blk = ids_s[cc * TCH:(cc + 1) * TCH]
        a0s.append(min(int(blk.min()), S - 8 * spc) if 8 * spc < S else 0)

    nsl = NCH * spc
    jgs, a_in_p0 = _passes(spc)
    nja = len(jgs[0]) if a_in_p0 else 0
    ACW = nsl * P
    XO = ACW
    WO = ACW + NTOK
    PKW = WO + nja * P
    NJB = KD - nja

    # Replicated weight layouts with contiguous per-partition runs.
    a_cat = lora_a.transpose(1, 0, 2).reshape(D, SR)                  # (D, S*R)
    b_stk = lora_b.reshape(SR, F)                                     # (S*R, F)
    # wk4[k, p, j, fi] = kernel[k*128+p, j*128+fi]
    wk4 = kernel_w.reshape(KD, P, KD, P).astype(BF16)
    wkb_l = np.ascontiguousarray(
        wk4[:, :, nja:, :].reshape(KD, P, NJB * P).transpose(1, 0, 2))
    bia_l = np.ascontiguousarray(bias.reshape(KD, P).T.astype(np.float32))

    # Per-(slab-row, band-slab) local adapter index: (o*128+p)//16
    adiv = (np.arange(spc)[None, :] * P + np.arange(P)[:, None]) // R  # (P, spc)

    in_maps = []
    for c in range(NCORES):
        lo = c * NTOK
        xs = xs_all[lo:lo + NTOK]                                     # (NTOK, D)
        ac_g = np.empty((D, nsl * P), dtype=BF16)
        bs_g = np.empty((nsl, P, F), dtype=BF16)
        msk_l = np.empty((P, spc, NTOK), dtype=BF16)
        for t in range(NCH):
            a0 = a0s[c * NCH + t]
            sr0 = a0 * R
            ac_g[:, (t * spc) * P:(t * spc + spc) * P] = \
                a_cat[:, sr0:sr0 + spc * P].astype(BF16)
            bs_g[t * spc:(t + 1) * spc] = \
                b_stk[sr0:sr0 + spc * P].reshape(spc, P, F).astype(BF16)
            lid = ids_s[lo + t * TCH: lo + (t + 1) * TCH] - a0        # (TCH,)
            msk_l[:, :, t * TCH:(t + 1) * TCH] = \
                (adiv[:, :, None] == lid[None, None, :]).astype(BF16)
        # Pack [A band | x^T | first W f-blocks] per D-slab k.
        pk_l = np.empty((P, KD, PKW), dtype=BF16)
        pk_l[:, :, :ACW] = ac_g.reshape(KD, P, ACW).transpose(1, 0, 2)
        pk_l[:, :, XO:WO] = \
            xs.T.reshape(KD, P, NTOK).transpose(1, 0, 2).astype(BF16)
        pk_l[:, :, WO:] = \
            wk4[:, :, :nja, :].reshape(KD, P, nja * P).transpose(1, 0, 2)
        bs_l = np.ascontiguousarray(bs_g.transpose(1, 0, 2))
        in_maps.append({
            "pk": np.ascontiguousarray(pk_l), "wkb": wkb_l, "bs": bs_l,
            "msk": np.ascontiguousarray(msk_l), "bia": bia_l,
        })

    nc = _get_nc(spc)
    res = run_bass_kernel_spmd(nc, in_maps, core_ids=list(range(NCORES)),
                               trace=TRACE)
    LAST_RESULTS = res
    LAST_IN_MAPS = in_maps
    LAST_NC = nc
    LAST_NS = spc

    out = np.empty((N, F), dtype=np.float32)
    for c in range(NCORES):
        # out_s[j, p, t] holds out^T for f = j*128+p -> reshape to (F, NTOK).
        core_out = res.results[c]["out_s"].reshape(F, NTOK).T
        out[perm[c * NTOK:(c + 1) * NTOK]] = core_out.astype(np.float32)
    return out


# revision 30
# speedup vs baseline: 1.2854x; 1.2854x over previous
"""LoRADense (per-token adapter routing) Bass kernel for 8 Trainium2 NeuronCores.

Math (reference):
    base  = x @ kernel + bias                      # (N, F)
    a     = lora_a[adapter_ids]                    # (N, D, R) gather
    b     = lora_b[adapter_ids]                    # (N, R, F) gather
    lr    = einsum('nd,ndr->nr', x, a)             # (N, R)
    delta = einsum('nr,nrf->nf', lr, b)            # (N, F)
    out   = base + delta

Strategy (v6):
  - GLOBAL sort of all 8192 tokens by adapter id on the host; core c gets the
    contiguous sorted run [1024c, 1024(c+1)).  Within a core, each 512-token
    chunk sees only ~5 consecutive adapter ids, so the host gathers, per
    (core, chunk), one 128-row band (8 adapters; spc slabs in general) of the
    concatenated LoRA factors, re-based so the device program is identical on
    every core (SPMD-safe).
  - Transposed compute: out^T[f, tok]; moving operand is always the token
    axis (512-wide chunks).
  - fp8 DoubleRow with residual compensation for the big contractions.  A
    DoubleRow matmul computes w0*m0 + w1*m1 per cell at 0.5 cycles/row, so:
      * "3-product" slab k (exact to ~1e-3): main  [Q;Qr] x [x8;x8]
        (Q=fp8(M), Qr=fp8(M-Q)) plus, per slab PAIR, one shared corrector
        [Q_k;Q_k1] x [xr8_k;xr8_k1] -> recovers x8@Q + x8@Qr + xr8@Q,
        i.e. x@M up to ~0.1% at 0.75x the bf16 cost.
      * "x-comp" slab k (cheap): [Q;Q] x [x8;xr8] -> (x8+xr8)@Q, leaving
        only the weight-quantization error (~0.7e-2 per slab) at 0.5x cost.
    The base GEMM uses x-comp on XC_KS slabs and 3-product on the rest;
    stage A (the LoRA lr) is all 3-product.  Measured end-to-end error
    ~1.5e-2 against the 2e-2 gate.  The LoRA delta path stays bf16.
  - stage A output is masked per (sr row, token) on DVE -> bf16 lrm; each
    out^T group accumulates base + B_band^T @ lrm in one PSUM group, then
    +bias fused with the f32->bf16 convert, DMA to DRAM.
  - k-major schedule in f-block passes sized to the 8 PSUM banks; pass 0
    carries stage A; per-k just-in-time DMA stream.
  - Host un-permutes rows and upcasts to f32.
"""

import numpy as np
import ml_dtypes

import concourse.bacc as bacc
import concourse.bass as bass
import concourse.mybir as mybir
import concourse.tile as tile
from concourse.bass_utils import run_bass_kernel_spmd

# Problem constants (hardcoded per harness contract).
N = 8192          # tokens
D = 1024          # input dim
F = 1024          # output features
R = 16            # lora rank
S = 64            # adapter slots
SR = S * R        # 1024
NCORES = 8
NTOK = N // NCORES            # 1024 tokens per core
P = 128                       # partitions
KD = D // P                   # 8 contraction slabs over D
TCH = 512                     # moving-operand token chunk
NCH = NTOK // TCH             # 2 chunks per core

N_XC = 4                      # base slabs using cheap x-comp fp8 (k < N_XC); even
assert N_XC % 2 == 0

BF16 = ml_dtypes.bfloat16
FP8 = ml_dtypes.float8_e4m3
DR = mybir.MatmulPerfMode.DoubleRow

# Toggles (test.py pokes these).
TRACE = False
LAST_RESULTS = None
LAST_IN_MAPS = None
LAST_NC = None
LAST_NS = None

JUNK = 24
FORCE_SPC = None  # testing hook
_NC_CACHE = {}


def _passes(spc):
    """f-block passes + whether stage A rides in pass 0, given PSUM budget 8."""
    n_lr = NCH * spc
    if n_lr <= 8 - NCH:  # room for at least one f-block next to the lr banks
        g0 = (8 - n_lr) // NCH
        jgs = [tuple(range(g0))]
        a_in_pass0 = True
    else:
        jgs = []
        a_in_pass0 = False
        g0 = 0
    j = g0
    while j < KD:
        # width-2 passes (last f-block alone) spread closers/out-DMAs evenly
        g = min(2, KD - 1 - j) if j < KD - 1 else 1
        g = max(1, g)
        jgs.append(tuple(range(j, j + g)))
        j += g
    return jgs, a_in_pass0


def _build_nc(spc):
    """Build the single-core Bass program (same program runs on all 8 cores).

    spc = LoRA slabs (128-row bands) per 512-token chunk; normally 1.
    """
    f32 = mybir.dt.float32
    bf16 = mybir.dt.bfloat16
    fp8 = mybir.dt.float8e4
    nsl = NCH * spc                 # total gathered slabs per core
    jgs, a_in_p0 = _passes(spc)
    nja = len(jgs[0]) if a_in_p0 else 0   # f-blocks in the k-stream W tensor
    njb = KD - nja

    nc = bacc.Bacc("TRN2", target_bir_lowering=False, debug=False)

    # DRAM I/O. Layouts are pre-packed on the host so every DMA is a plain
    # contiguous [partition, free...] copy.
    # xl:  [d_p, k, {x8, xr8}, tok]
    # ap8: [d_p, k, {A8, Ar8}, sr_loc]
    # wpa/wpb: [d_p, k, {W8, Wr8|W8}, j, f_i]  (pass-0 f-blocks / the rest)
    xl = nc.dram_tensor("xl", [P, KD, 2, NTOK], fp8, kind="ExternalInput")
    ap8 = nc.dram_tensor("ap8", [P, KD, 2, nsl * P], fp8, kind="ExternalInput")
    wpa = nc.dram_tensor("wpa", [P, KD, 2, nja * P], fp8, kind="ExternalInput")
    wpb = nc.dram_tensor("wpb", [P, KD, 2, njb * P], fp8, kind="ExternalInput")
    bs = nc.dram_tensor("bs", [P, nsl, F], bf16, kind="ExternalInput")
    msk = nc.dram_tensor("msk", [P, spc, NTOK], bf16, kind="ExternalInput")
    bia = nc.dram_tensor("bia", [P, KD], f32, kind="ExternalInput")
    out_s = nc.dram_tensor("out_s", [KD, P, NTOK], bf16, kind="ExternalOutput")

    with tile.TileContext(nc) as tc:
        with (
            tc.tile_pool(name="const", bufs=1) as cpool,
            tc.tile_pool(name="work", bufs=4) as wpool,
            tc.tile_pool(name="accp", bufs=8, space="PSUM") as accp,
        ):
            # Just-in-time DMA stream: per D-slab k, the A band, the x pair
            # and the pass-0 W f-blocks land together.
            ap8_sb = cpool.tile([P, KD, 2, nsl * P], fp8)
            xl_sb = cpool.tile([P, KD, 2, NTOK], fp8)
            wpa_sb = cpool.tile([P, KD, 2, nja * P], fp8)
            for k in range(0, KD, 2):
                nc.sync.dma_start(ap8_sb[:, k:k + 2], ap8[:, k:k + 2])
                if k == 0:
                    nc.sync.dma_start(xl_sb[:, 0], xl[:, 0])
                    nc.sync.dma_start(xl_sb[:, 1], xl[:, 1])
                else:
                    nc.sync.dma_start(xl_sb[:, k:k + 2], xl[:, k:k + 2])
                nc.sync.dma_start(wpa_sb[:, k:k + 2], wpa[:, k:k + 2])
            msk_sb = cpool.tile([P, spc, NTOK], bf16)
            nc.sync.dma_start(msk_sb[:], msk[:])
            bia_sb = cpool.tile([P, KD], f32)
            nc.sync.dma_start(bia_sb[:], bia[:])
            bs_sb = cpool.tile([P, nsl, F], bf16)
            nc.sync.dma_start(bs_sb[:], bs[:])
            wpb_sb = cpool.tile([P, KD, 2, njb * P], fp8)
            for k in range(0, KD, 2):
                nc.sync.dma_start(wpb_sb[:, k:k + 2], wpb[:, k:k + 2])

            def wblk(k, j):
                # [P, 2, 128] {W8, layer1} block for (k-slab, f-block j)
                if j < nja:
                    return wpa_sb[:, k, :, j * P:(j + 1) * P]
                jj = j - nja
                return wpb_sb[:, k, :, jj * P:(jj + 1) * P]

            def wblk_w8pair(k, j):
                # [P, 2(k,k+1), 128] of layer-0 (W8) for the shared corrector
                if j < nja:
                    return wpa_sb[:, k:k + 2, 0, j * P:(j + 1) * P]
                jj = j - nja
                return wpb_sb[:, k:k + 2, 0, jj * P:(jj + 1) * P]

            def x8_dup(k, tok):
                # [P, 2, TCH] broadcast of the x8 layer (stride-0 pair dim)
                return xl_sb[:, k, 0, tok].unsqueeze(1).broadcast_to(
                    (P, 2, TCH))

            # Masked low-rank activations, bf16: [sr_p, chunk-band, tok]
            lrm_sb = cpool.tile([P, spc, NTOK], bf16)

            # Warm-up: keep the PE busy (and the HAM clock-gate ramping)
            # while the first input packs are still in flight.
            junk_sb = cpool.tile([P, P], bf16)
            nc.vector.memset(junk_sb[:], 0.0)
            # Preload the ACT function table off the critical path.
            atw_sb = cpool.tile([P, 8], bf16)
            nc.scalar.activation(atw_sb[:], junk_sb[:, :8],
                                 mybir.ActivationFunctionType.Identity)
            jp = accp.tile([P, TCH], mybir.dt.float32, tag="acc", name="jp")
            for w in range(JUNK):
                nc.tensor.matmul(
                    jp[:, :P], junk_sb[:], junk_sb[:],
                    start=True, stop=True,
                )

            def stage_a(t, o, k, ps):
                # 3-product compensated lr: main [A8;Ar8]x[x8;x8] per slab,
                # shared corrector [A8_k;A8_k1]x[xr8_k;xr8_k1] per pair.
                tok = slice(t * TCH, (t + 1) * TCH)
                band = slice((t * spc + o) * P, (t * spc + o + 1) * P)
                nc.tensor.matmul(
                    ps[:], ap8_sb[:, k, :, band], x8_dup(k, tok),
                    start=(k == 0), stop=False, perf_mode=DR,
                )
                if k % 2 == 1:
                    nc.tensor.matmul(
                        ps[:], ap8_sb[:, k - 1:k + 1, 0, band],
                        xl_sb[:, k - 1:k + 1, 1, tok],
                        start=False, stop=(k == KD - 1), perf_mode=DR,
                    )
                if k == KD - 1:
                    # msk[p, o, tok] = (lid[tok] == (o*128+p)//16), host-built
                    nc.vector.tensor_tensor(
                        lrm_sb[:, o, tok],
                        ps[:],
                        msk_sb[:, o, tok],
                        mybir.AluOpType.mult,
                    )

            def base_mm(t, j, k, po):
                # x-comp slab: [W8;W8]x[x8;xr8]; 3-product slab: main +
                # (at odd k) the pair's shared corrector.
                tok = slice(t * TCH, (t + 1) * TCH)
                if k < N_XC:
                    nc.tensor.matmul(
                        po[:], wblk(k, j), xl_sb[:, k, :, tok],
                        start=(k == 0), stop=False, perf_mode=DR,
                    )
                else:
                    nc.tensor.matmul(
                        po[:], wblk(k, j), x8_dup(k, tok),
                        start=(k == 0), stop=False, perf_mode=DR,
                    )
                    if k % 2 == 1:
                        nc.tensor.matmul(
                            po[:], wblk_w8pair(k - 1, j),
                            xl_sb[:, k - 1:k + 1, 1, tok],
                            start=False, stop=False, perf_mode=DR,
                        )

            ob_sb = cpool.tile([P, KD, NTOK], bf16)

            def close_group(t, j, po):
                tok = slice(t * TCH, (t + 1) * TCH)
                for o in range(spc):
                    nc.tensor.matmul(
                        po[:],
                        bs_sb[:, t * spc + o, j * P:(j + 1) * P],
                        lrm_sb[:, o, tok],
                        start=False,
                        stop=(o == spc - 1),
                    )
                nc.any.tensor_scalar_add(ob_sb[:, j, tok], po[:],
                                         bia_sb[:, j:j + 1])
                if j == KD - 1:
                    # last f-block: per-chunk DMA so the first half overlaps
                    # the final chunk's close + convert
                    nc.sync.dma_start(out_s[j, :, tok], ob_sb[:, j, tok])

            run_a = a_in_p0
            if not a_in_p0:
                # Fallback: sequential stage A before the f-block passes.
                for t in range(NCH):
                    for o in range(spc):
                        ps = accp.tile([P, TCH], mybir.dt.float32, tag="acc",
                                       name=f"lr_{t}_{o}")
                        for k in range(KD):
                            stage_a(t, o, k, ps)

            for gi, jg in enumerate(jgs):
                last = gi == len(jgs) - 1
                pos = {}
                lrs = {}
                for t in range(NCH):
                    for j in jg:
                        pos[(t, j)] = accp.tile(
                            [P, TCH], mybir.dt.float32, tag="acc",
                            name=f"po_{t}_{j}")
                    if gi == 0 and run_a:
                        for o in range(spc):
                            lrs[(t, o)] = accp.tile(
                                [P, TCH], mybir.dt.float32, tag="acc",
                                name=f"lr_{t}_{o}")
                if last:
                    # t-major: the first chunk's close/convert/DMA overlaps
                    # the second chunk's matmuls, shortening the tail.
                    for t in range(NCH):
                        for k in range(KD):
                            for j in jg:
                                base_mm(t, j, k, pos[(t, j)])
                        for j in jg:
                            close_group(t, j, pos[(t, j)])
                    continue
                for k in range(KD):
                    for t in range(NCH):
                        if gi == 0 and run_a:
                            for o in range(spc):
                                stage_a(t, o, k, lrs[(t, o)])
                        for j in jg:
                            base_mm(t, j, k, pos[(t, j)])
                for t in range(NCH):
                    for j in jg:
                        close_group(t, j, pos[(t, j)])
                nc.sync.dma_start(
                    out_s[jg[0]:jg[-1] + 1].transpose([1, 0, 2]),
                    ob_sb[:, jg[0]:jg[-1] + 1])

    nc.compile()
    return nc


def _get_nc(spc):
    key = (spc, JUNK, N_XC)
    if key not in _NC_CACHE:
        _NC_CACHE[key] = _build_nc(spc)
    return _NC_CACHE[key]


def _fp8_pair(m):
    """fp8 value + fp8 residual of a float32 array."""
    q = m.astype(FP8)
    r = (m - q.astype(np.float32)).astype(FP8)
    return q, r


def kernel(x, adapter_ids, kernel, bias, lora_a, lora_b):
    global LAST_RESULTS, LAST_IN_MAPS, LAST_NC, LAST_NS
    x = np.ascontiguousarray(np.asarray(x, dtype=np.float32))
    adapter_ids = np.asarray(adapter_ids)
    kernel_w = np.asarray(kernel, dtype=np.float32)
    bias = np.asarray(bias, dtype=np.float32)
    lora_a = np.asarray(lora_a, dtype=np.float32)
    lora_b = np.asarray(lora_b, dtype=np.float32)
    ids = adapter_ids.astype(np.int64)

    # Global stable sort by adapter id; each core gets a contiguous run.
    perm = np.argsort(ids, kind="stable")
    ids_s = ids[perm]
    xs_all = x[perm]

    # Per-(core, chunk) adapter band [a0, a0 + 8*spc).
    spans = []
    for cc in range(NCORES * NCH):
        blk = ids_s[cc * TCH:(cc + 1) * TCH]
        spans.append(int(blk.max()) - int(blk.min()) + 1)
    spc = FORCE_SPC or max(1, int(np.ceil(max(spans) / 8)))
    a0s = []
    for cc in range(NCORES * NCH):
        blk = ids_s[cc * TCH:(cc + 1) * TCH]
        a0s.append(min(int(blk.min()), S - 8 * spc) if 8 * spc < S else 0)

    nsl = NCH * spc
    jgs, a_in_p0 = _passes(spc)
    nja = len(jgs[0]) if a_in_p0 else 0
    njb = KD - nja

    # Replicated weight layouts with contiguous per-partition runs.
    a_cat = lora_a.transpose(1, 0, 2).reshape(D, SR)                  # (D, S*R)
    b_stk = lora_b.reshape(SR, F)                                     # (S*R, F)
    A8, Ar8 = _fp8_pair(a_cat)
    W8, Wr8 = _fp8_pair(kernel_w)
    # wp[p, k, l, j, fi]; layer1 = Wr8 for 3-product slabs, W8 for x-comp.
    wp = np.empty((P, KD, 2, KD, P), dtype=FP8)
    wp[:, :, 0] = W8.reshape(KD, P, KD, P).transpose(1, 0, 2, 3)
    l1 = Wr8.reshape(KD, P, KD, P).copy()
    l1[:N_XC] = W8.reshape(KD, P, KD, P)[:N_XC]
    wp[:, :, 1] = l1.transpose(1, 0, 2, 3)
    wpa_l = np.ascontiguousarray(
        wp[:, :, :, :nja].reshape(P, KD, 2, nja * P))
    wpb_l = np.ascontiguousarray(
        wp[:, :, :, nja:].reshape(P, KD, 2, njb * P))
    bia_l = np.ascontiguousarray(bias.reshape(KD, P).T.astype(np.float32))

    # Per-(slab-row, band-slab) local adapter index: (o*128+p)//16
    adiv = (np.arange(spc)[None, :] * P + np.arange(P)[:, None]) // R  # (P, spc)

    in_maps = []
    for c in range(NCORES):
        lo = c * NTOK
        xs = xs_all[lo:lo + NTOK]                                     # (NTOK, D)
        x8, xr8 = _fp8_pair(xs)
        xl_l = np.empty((P, KD, 2, NTOK), dtype=FP8)
        xl_l[:, :, 0] = x8.T.reshape(KD, P, NTOK).transpose(1, 0, 2)
        xl_l[:, :, 1] = xr8.T.reshape(KD, P, NTOK).transpose(1, 0, 2)
        ap_g = np.empty((P, KD, 2, nsl * P), dtype=FP8)
        bs_g = np.empty((nsl, P, F), dtype=BF16)
        msk_l = np.empty((P, spc, NTOK), dtype=BF16)
        for t in range(NCH):
            a0 = a0s[c * NCH + t]
            sr0 = a0 * R
            cols = slice(sr0, sr0 + spc * P)
            ap_g[:, :, 0, t * spc * P:(t * spc + spc) * P] = \
                A8[:, cols].reshape(KD, P, spc * P).transpose(1, 0, 2)
            ap_g[:, :, 1, t * spc * P:(t * spc + spc) * P] = \
                Ar8[:, cols].reshape(KD, P, spc * P).transpose(1, 0, 2)
            bs_g[t * spc:(t + 1) * spc] = \
                b_stk[cols].reshape(spc, P, F).astype(BF16)
            lid = ids_s[lo + t * TCH: lo + (t + 1) * TCH] - a0        # (TCH,)
            msk_l[:, :, t * TCH:(t + 1) * TCH] = \
                (adiv[:, :, None] == lid[None, None, :]).astype(BF16)
        bs_l = np.ascontiguousarray(bs_g.transpose(1, 0, 2))
        in_maps.append({
            "xl": np.ascontiguousarray(xl_l), "ap8": np.ascontiguousarray(ap_g),
            "wpa": wpa_l, "wpb": wpb_l, "bs": bs_l,
            "msk": np.ascontiguousarray(msk_l), "bia": bia_l,
        })

    nc = _get_nc(spc)
    res = run_bass_kernel_spmd(nc, in_maps, core_ids=list(range(NCORES)),
                               trace=TRACE)
    LAST_RESULTS = res
    LAST_IN_MAPS = in_maps
    LAST_NC = nc
    LAST_NS = spc

    out = np.empty((N, F), dtype=np.float32)
    for c in range(NCORES):
        # out_s[j, p, t] holds out^T for f = j*128+p -> reshape to (F, NTOK).
        core_out = res.results[c]["out_s"].reshape(F, NTOK).T
        out[perm[c * NTOK:(c + 1) * NTOK]] = core_out.astype(np.float32)
    return out


# revision 32
# speedup vs baseline: 1.3069x; 1.0167x over previous
"""LoRADense (per-token adapter routing) Bass kernel for 8 Trainium2 NeuronCores.

Math (reference):
    base  = x @ kernel + bias                      # (N, F)
    a     = lora_a[adapter_ids]                    # (N, D, R) gather
    b     = lora_b[adapter_ids]                    # (N, R, F) gather
    lr    = einsum('nd,ndr->nr', x, a)             # (N, R)
    delta = einsum('nr,nrf->nf', lr, b)            # (N, F)
    out   = base + delta

Strategy (v6):
  - GLOBAL sort of all 8192 tokens by adapter id on the host; core c gets the
    contiguous sorted run [1024c, 1024(c+1)).  Within a core, each 512-token
    chunk sees only ~5 consecutive adapter ids, so the host gathers, per
    (core, chunk), one 128-row band (8 adapters; spc slabs in general) of the
    concatenated LoRA factors, re-based so the device program is identical on
    every core (SPMD-safe).
  - Transposed compute: out^T[f, tok]; moving operand is always the token
    axis (512-wide chunks).
  - fp8 DoubleRow with residual compensation for the big contractions.  A
    DoubleRow matmul computes w0*m0 + w1*m1 per cell at 0.5 cycles/row, so:
      * "3-product" slab k (exact to ~1e-3): main  [Q;Qr] x [x8;x8]
        (Q=fp8(M), Qr=fp8(M-Q)) plus, per slab PAIR, one shared corrector
        [Q_k;Q_k1] x [xr8_k;xr8_k1] -> recovers x8@Q + x8@Qr + xr8@Q,
        i.e. x@M up to ~0.1% at 0.75x the bf16 cost.
      * "x-comp" slab k (cheap): [Q;Q] x [x8;xr8] -> (x8+xr8)@Q, leaving
        only the weight-quantization error (~0.7e-2 per slab) at 0.5x cost.
    The base GEMM uses x-comp on XC_KS slabs and 3-product on the rest;
    stage A (the LoRA lr) is all 3-product.  Measured end-to-end error
    ~1.5e-2 against the 2e-2 gate.  The LoRA delta path stays bf16.
  - stage A output is masked per (sr row, token) on DVE -> bf16 lrm; each
    out^T group accumulates base + B_band^T @ lrm in one PSUM group, then
    +bias fused with the f32->bf16 convert, DMA to DRAM.
  - k-major schedule in f-block passes sized to the 8 PSUM banks; pass 0
    carries stage A; per-k just-in-time DMA stream.
  - Host un-permutes rows and upcasts to f32.
"""

import numpy as np
import ml_dtypes

import concourse.bacc as bacc
import concourse.bass as bass
import concourse.mybir as mybir
import concourse.tile as tile
from concourse.bass_utils import run_bass_kernel_spmd

# Problem constants (hardcoded per harness contract).
N = 8192          # tokens
D = 1024          # input dim
F = 1024          # output features
R = 16            # lora rank
S = 64            # adapter slots
SR = S * R        # 1024
NCORES = 8
NTOK = N // NCORES            # 1024 tokens per core
P = 128                       # partitions
KD = D // P                   # 8 contraction slabs over D
TCH = 512                     # moving-operand token chunk
NCH = NTOK // TCH             # 2 chunks per core

N_XC = 4                      # base slabs using cheap x-comp fp8 (k < N_XC); even
assert N_XC % 2 == 0

BF16 = ml_dtypes.bfloat16
FP8 = ml_dtypes.float8_e4m3
DR = mybir.MatmulPerfMode.DoubleRow

# Toggles (test.py pokes these).
TRACE = False
LAST_RESULTS = None
LAST_IN_MAPS = None
LAST_NC = None
LAST_NS = None

JUNK = 24
FORCE_SPC = None  # testing hook
_NC_CACHE = {}


def _passes(spc):
    """f-block passes + whether stage A rides in pass 0, given PSUM budget 8."""
    n_lr = NCH * spc
    if n_lr <= 8 - NCH:  # room for at least one f-block next to the lr banks
        g0 = (8 - n_lr) // NCH
        jgs = [tuple(range(g0))]
        a_in_pass0 = True
    else:
        jgs = []
        a_in_pass0 = False
        g0 = 0
    j = g0
    while j < KD:
        # width-2 passes (last f-block alone) spread closers/out-DMAs evenly
        g = min(2, KD - 1 - j) if j < KD - 1 else 1
        g = max(1, g)
        jgs.append(tuple(range(j, j + g)))
        j += g
    return jgs, a_in_pass0


def _build_nc(spc):
    """Build the single-core Bass program (same program runs on all 8 cores).

    spc = LoRA slabs (128-row bands) per 512-token chunk; normally 1.
    """
    f32 = mybir.dt.float32
    bf16 = mybir.dt.bfloat16
    fp8 = mybir.dt.float8e4
    nsl = NCH * spc                 # total gathered slabs per core
    jgs, a_in_p0 = _passes(spc)
    nja = len(jgs[0]) if a_in_p0 else 0   # f-blocks in the k-stream W tensor
    njb = KD - nja

    nc = bacc.Bacc("TRN2", target_bir_lowering=False, debug=False)

    # DRAM I/O. Layouts are pre-packed on the host so every DMA is a plain
    # contiguous [partition, free...] copy.
    # xl:  [d_p, k, {x8, xr8}, tok]
    # ap8: [d_p, k, {A8, Ar8}, sr_loc]
    # w2a/w2b: [d_p, i, {W8, Wr8}, j, f_i]   (3-product slabs k=i)
    # wxa/wxb: [d_p, i, j, f_i]              (x-comp slabs k=N3L+i, W8 only)
    n3l = KD - N_XC
    xl = nc.dram_tensor("xl", [P, KD, 2, NTOK], fp8, kind="ExternalInput")
    ap8 = nc.dram_tensor("ap8", [P, KD, 2, nsl * P], fp8, kind="ExternalInput")
    w2a = nc.dram_tensor("w2a", [P, n3l, 2, nja * P], fp8, kind="ExternalInput")
    w2b = nc.dram_tensor("w2b", [P, n3l, 2, njb * P], fp8, kind="ExternalInput")
    wxa = nc.dram_tensor("wxa", [P, N_XC, nja * P], fp8, kind="ExternalInput")
    wxb = nc.dram_tensor("wxb", [P, N_XC, njb * P], fp8, kind="ExternalInput")
    bs = nc.dram_tensor("bs", [P, nsl, F], bf16, kind="ExternalInput")
    msk = nc.dram_tensor("msk", [P, spc, NTOK], bf16, kind="ExternalInput")
    bia = nc.dram_tensor("bia", [P, KD], f32, kind="ExternalInput")
    out_s = nc.dram_tensor("out_s", [KD, P, NTOK], bf16, kind="ExternalOutput")

    with tile.TileContext(nc) as tc:
        with (
            tc.tile_pool(name="const", bufs=1) as cpool,
            tc.tile_pool(name="work", bufs=4) as wpool,
            tc.tile_pool(name="accp", bufs=8, space="PSUM") as accp,
        ):
            # Just-in-time DMA stream: per slab PAIR, the A band layers, the
            # x layers and the pass-0 W blocks land together.
            ap8_sb = cpool.tile([P, KD, 2, nsl * P], fp8)
            xl_sb = cpool.tile([P, KD, 2, NTOK], fp8)
            w2a_sb = cpool.tile([P, n3l, 2, nja * P], fp8)
            wxa_sb = cpool.tile([P, N_XC, nja * P], fp8)
            for k in range(0, KD, 2):
                nc.sync.dma_start(ap8_sb[:, k:k + 2], ap8[:, k:k + 2])
                if k == 0:
                    nc.sync.dma_start(xl_sb[:, 0], xl[:, 0])
                    nc.sync.dma_start(xl_sb[:, 1], xl[:, 1])
                else:
                    nc.sync.dma_start(xl_sb[:, k:k + 2], xl[:, k:k + 2])
                if k < n3l:
                    nc.sync.dma_start(w2a_sb[:, k:k + 2], w2a[:, k:k + 2])
                else:
                    i = k - n3l
                    nc.sync.dma_start(wxa_sb[:, i:i + 2], wxa[:, i:i + 2])
            msk_sb = cpool.tile([P, spc, NTOK], bf16)
            nc.sync.dma_start(msk_sb[:], msk[:])
            bia_sb = cpool.tile([P, KD], f32)
            nc.sync.dma_start(bia_sb[:], bia[:])
            bs_sb = cpool.tile([P, nsl, F], bf16)
            nc.sync.dma_start(bs_sb[:], bs[:])
            w2b_sb = cpool.tile([P, n3l, 2, njb * P], fp8)
            wxb_sb = cpool.tile([P, N_XC, njb * P], fp8)
            for k in range(0, n3l, 2):
                nc.sync.dma_start(w2b_sb[:, k:k + 2], w2b[:, k:k + 2])
            for i in range(0, N_XC, 2):
                nc.sync.dma_start(wxb_sb[:, i:i + 2], wxb[:, i:i + 2])

            def w3pair(kp, layer, j):
                # [P, 2(k pair), 128] of W8 (layer 0) / Wr8 (layer 1)
                if j < nja:
                    return w2a_sb[:, kp:kp + 2, layer, j * P:(j + 1) * P]
                jj = j - nja
                return w2b_sb[:, kp:kp + 2, layer, jj * P:(jj + 1) * P]

            def wxpair(kp, j):
                i = kp - n3l
                if j < nja:
                    return wxa_sb[:, i:i + 2, j * P:(j + 1) * P]
                jj = j - nja
                return wxb_sb[:, i:i + 2, jj * P:(jj + 1) * P]

            # Masked low-rank activations, bf16: [sr_p, chunk-band, tok]
            lrm_sb = cpool.tile([P, spc, NTOK], bf16)

            # Warm-up: keep the PE busy (and the HAM clock-gate ramping)
            # while the first input packs are still in flight.
            junk_sb = cpool.tile([P, P], bf16)
            nc.vector.memset(junk_sb[:], 0.0)
            # Preload the ACT function table off the critical path.
            atw_sb = cpool.tile([P, 8], bf16)
            nc.scalar.activation(atw_sb[:], junk_sb[:, :8],
                                 mybir.ActivationFunctionType.Identity)
            jp = accp.tile([P, TCH], mybir.dt.float32, tag="acc", name="jp")
            for w in range(JUNK):
                nc.tensor.matmul(
                    jp[:, :P], junk_sb[:], junk_sb[:],
                    start=True, stop=True,
                )

            def stage_a(t, o, kp, ps):
                # 3-product compensated lr over slab pair (kp, kp+1):
                #   M1 [A8;A8'] x [x8;x8'] + M2 [Ar8;Ar8'] x [x8;x8']
                # + M3 [A8;A8'] x [xr8;xr8']   (drops only xr*Ar terms)
                tok = slice(t * TCH, (t + 1) * TCH)
                band = slice((t * spc + o) * P, (t * spc + o + 1) * P)
                x8p = xl_sb[:, kp:kp + 2, 0, tok]
                xrp = xl_sb[:, kp:kp + 2, 1, tok]
                nc.tensor.matmul(
                    ps[:], ap8_sb[:, kp:kp + 2, 0, band], x8p,
                    start=(kp == 0), stop=False, perf_mode=DR,
                )
                nc.tensor.matmul(
                    ps[:], ap8_sb[:, kp:kp + 2, 1, band], x8p,
                    start=False, stop=False, perf_mode=DR,
                )
                nc.tensor.matmul(
                    ps[:], ap8_sb[:, kp:kp + 2, 0, band], xrp,
                    start=False, stop=(kp == KD - 2), perf_mode=DR,
                )
                if kp == KD - 2:
                    # msk[p, o, tok] = (lid[tok] == (o*128+p)//16), host-built
                    nc.vector.tensor_tensor(
                        lrm_sb[:, o, tok],
                        ps[:],
                        msk_sb[:, o, tok],
                        mybir.AluOpType.mult,
                    )

            def base_mm(t, j, kp, po):
                # slab pair (kp, kp+1): 3-product slabs get M1+M2+M3; x-comp
                # slabs get M1+M3 (leaving only the W-quantization error).
                tok = slice(t * TCH, (t + 1) * TCH)
                x8p = xl_sb[:, kp:kp + 2, 0, tok]
                xrp = xl_sb[:, kp:kp + 2, 1, tok]
                if kp < n3l:
                    nc.tensor.matmul(
                        po[:], w3pair(kp, 0, j), x8p,
                        start=(kp == 0), stop=False, perf_mode=DR,
                    )
                    nc.tensor.matmul(
                        po[:], w3pair(kp, 1, j), x8p,
                        start=False, stop=False, perf_mode=DR,
                    )
                    nc.tensor.matmul(
                        po[:], w3pair(kp, 0, j), xrp,
                        start=False, stop=False, perf_mode=DR,
                    )
                else:
                    nc.tensor.matmul(
                        po[:], wxpair(kp, j), x8p,
                        start=(kp == 0), stop=False, perf_mode=DR,
                    )
                    nc.tensor.matmul(
                        po[:], wxpair(kp, j), xrp,
                        start=False, stop=False, perf_mode=DR,
                    )

            ob_sb = cpool.tile([P, KD, NTOK], bf16)

            def close_group(t, j, po):
                tok = slice(t * TCH, (t + 1) * TCH)
                for o in range(spc):
                    nc.tensor.matmul(
                        po[:],
                        bs_sb[:, t * spc + o, j * P:(j + 1) * P],
                        lrm_sb[:, o, tok],
                        start=False,
                        stop=(o == spc - 1),
                    )
                nc.any.tensor_scalar_add(ob_sb[:, j, tok], po[:],
                                         bia_sb[:, j:j + 1])
                if j == KD - 1:
                    # last f-block: per-chunk DMA so the first half overlaps
                    # the final chunk's close + convert
                    nc.sync.dma_start(out_s[j, :, tok], ob_sb[:, j, tok])

            run_a = a_in_p0
            if not a_in_p0:
                # Fallback: sequential stage A before the f-block passes.
                for t in range(NCH):
                    for o in range(spc):
                        ps = accp.tile([P, TCH], mybir.dt.float32, tag="acc",
                                       name=f"lr_{t}_{o}")
                        for kp in range(0, KD, 2):
                            stage_a(t, o, kp, ps)

            for gi, jg in enumerate(jgs):
                last = gi == len(jgs) - 1
                pos = {}
                lrs = {}
                for t in range(NCH):
                    for j in jg:
                        pos[(t, j)] = accp.tile(
                            [P, TCH], mybir.dt.float32, tag="acc",
                            name=f"po_{t}_{j}")
                    if gi == 0 and run_a:
                        for o in range(spc):
                            lrs[(t, o)] = accp.tile(
                                [P, TCH], mybir.dt.float32, tag="acc",
                                name=f"lr_{t}_{o}")
                if last:
                    # t-major: the first chunk's close/convert/DMA overlaps
                    # the second chunk's matmuls, shortening the tail.
                    for t in range(NCH):
                        for kp in range(0, KD, 2):
                            for j in jg:
                                base_mm(t, j, kp, pos[(t, j)])
                        for j in jg:
                            close_group(t, j, pos[(t, j)])
                    continue
                for kp in range(0, KD, 2):
                    for t in range(NCH):
                        if gi == 0 and run_a:
                            for o in range(spc):
                                stage_a(t, o, kp, lrs[(t, o)])
                        for j in jg:
                            base_mm(t, j, kp, pos[(t, j)])
                for t in range(NCH):
                    for j in jg:
                        close_group(t, j, pos[(t, j)])
                nc.sync.dma_start(
                    out_s[jg[0]:jg[-1] + 1].transpose([1, 0, 2]),
                    ob_sb[:, jg[0]:jg[-1] + 1])

    nc.compile()
    return nc


def _get_nc(spc):
    key = (spc, JUNK, N_XC)
    if key not in _NC_CACHE:
        _NC_CACHE[key] = _build_nc(spc)
    return _NC_CACHE[key]


def _fp8_pair(m):
    """fp8 value + fp8 residual of a float32 array."""
    q = m.astype(FP8)
    r = (m - q.astype(np.float32)).astype(FP8)
    return q, r


def kernel(x, adapter_ids, kernel, bias, lora_a, lora_b):
    global LAST_RESULTS, LAST_IN_MAPS, LAST_NC, LAST_NS
    x = np.ascontiguousarray(np.asarray(x, dtype=np.float32))
    adapter_ids = np.asarray(adapter_ids)
    kernel_w = np.asarray(kernel, dtype=np.float32)
    bias = np.asarray(bias, dtype=np.float32)
    lora_a = np.asarray(lora_a, dtype=np.float32)
    lora_b = np.asarray(lora_b, dtype=np.float32)
    ids = adapter_ids.astype(np.int64)

    # Global stable sort by adapter id; each core gets a contiguous run.
    perm = np.argsort(ids, kind="stable")
    ids_s = ids[perm]
    xs_all = x[perm]

    # Per-(core, chunk) adapter band [a0, a0 + 8*spc).
    spans = []
    for cc in range(NCORES * NCH):
        blk = ids_s[cc * TCH:(cc + 1) * TCH]
        spans.append(int(blk.max()) - int(blk.min()) + 1)
    spc = FORCE_SPC or max(1, int(np.ceil(max(spans) / 8)))
    a0s = []
    for cc in range(NCORES * NCH):
        blk = ids_s[cc * TCH:(cc + 1) * TCH]
        a0s.append(min(int(blk.min()), S - 8 * spc) if 8 * spc < S else 0)

    nsl = NCH * spc
    jgs, a_in_p0 = _passes(spc)
    nja = len(jgs[0]) if a_in_p0 else 0
    njb = KD - nja

    # Replicated weight layouts with contiguous per-partition runs.
    a_cat = lora_a.transpose(1, 0, 2).reshape(D, SR)                  # (D, S*R)
    b_stk = lora_b.reshape(SR, F)                                     # (S*R, F)
    A8, Ar8 = _fp8_pair(a_cat)
    W8, Wr8 = _fp8_pair(kernel_w)
    n3l = KD - N_XC
    w8r = W8.reshape(KD, P, KD, P).transpose(1, 0, 2, 3)   # [P, k, j, fi]
    wrr = Wr8.reshape(KD, P, KD, P).transpose(1, 0, 2, 3)
    w2 = np.stack([w8r[:, :n3l], wrr[:, :n3l]], axis=2)    # [P, i, 2, j, fi]
    w2a_l = np.ascontiguousarray(w2[:, :, :, :nja].reshape(P, n3l, 2, nja * P))
    w2b_l = np.ascontiguousarray(w2[:, :, :, nja:].reshape(P, n3l, 2, njb * P))
    wxa_l = np.ascontiguousarray(
        w8r[:, n3l:, :nja].reshape(P, N_XC, nja * P))
    wxb_l = np.ascontiguousarray(
        w8r[:, n3l:, nja:].reshape(P, N_XC, njb * P))
    bia_l = np.ascontiguousarray(bias.reshape(KD, P).T.astype(np.float32))

    # Per-(slab-row, band-slab) local adapter index: (o*128+p)//16
    adiv = (np.arange(spc)[None, :] * P + np.arange(P)[:, None]) // R  # (P, spc)

    in_maps = []
    for c in range(NCORES):
        lo = c * NTOK
        xs = xs_all[lo:lo + NTOK]                                     # (NTOK, D)
        x8, xr8 = _fp8_pair(xs)
        xl_l = np.empty((P, KD, 2, NTOK), dtype=FP8)
        xl_l[:, :, 0] = x8.T.reshape(KD, P, NTOK).transpose(1, 0, 2)
        xl_l[:, :, 1] = xr8.T.reshape(KD, P, NTOK).transpose(1, 0, 2)
        ap_g = np.empty((P, KD, 2, nsl * P), dtype=FP8)
        bs_g = np.empty((nsl, P, F), dtype=BF16)
        msk_l = np.empty((P, spc, NTOK), dtype=BF16)
        for t in range(NCH):
            a0 = a0s[c * NCH + t]
            sr0 = a0 * R
            cols = slice(sr0, sr0 + spc * P)
            ap_g[:, :, 0, t * spc * P:(t * spc + spc) * P] = \
                A8[:, cols].reshape(KD, P, spc * P).transpose(1, 0, 2)
            ap_g[:, :, 1, t * spc * P:(t * spc + spc) * P] = \
                Ar8[:, cols].reshape(KD, P, spc * P).transpose(1, 0, 2)
            bs_g[t * spc:(t + 1) * spc] = \
                b_stk[cols].reshape(spc, P, F).astype(BF16)
            lid = ids_s[lo + t * TCH: lo + (t + 1) * TCH] - a0        # (TCH,)
            msk_l[:, :, t * TCH:(t + 1) * TCH] = \
                (adiv[:, :, None] == lid[None, None, :]).astype(BF16)
        bs_l = np.ascontiguousarray(bs_g.transpose(1, 0, 2))
        in_maps.append({
            "xl": np.ascontiguousarray(xl_l), "ap8": np.ascontiguousarray(ap_g),
            "w2a": w2a_l, "w2b": w2b_l, "wxa": wxa_l, "wxb": wxb_l,
            "bs": bs_l, "msk": np.ascontiguousarray(msk_l), "bia": bia_l,
        })

    nc = _get_nc(spc)
    res = run_bass_kernel_spmd(nc, in_maps, core_ids=list(range(NCORES)),
                               trace=TRACE)
    LAST_RESULTS = res
    LAST_IN_MAPS = in_maps
    LAST_NC = nc
    LAST_NS = spc

    out = np.empty((N, F), dtype=np.float32)
    for c in range(NCORES):
        # out_s[j, p, t] holds out^T for f = j*128+p -> reshape to (F, NTOK).
        core_out = res.results[c]["out_s"].reshape(F, NTOK).T
        out[perm[c * NTOK:(c + 1) * NTOK]] = core_out.astype(np.float32)
    return out
